# revision 1
# baseline (speedup 1.0000x reference)
"""Multi-headed self-attention (S=2048, D=1024, H=16) on 8 trn2 NeuronCores.

Sharding: tensor-parallel over heads (2 heads/core). Each core computes
qkv for its heads in transposed layout (so the softmaxed probabilities
feed the ctx matmul without a transpose), uses a no-max base-2 softmax
(2^s / sum 2^s == 2^(s-max) / sum 2^(s-max), with the denominator from
a fused ones-column in v and broadcast via a K=1 outer-product matmul),
then four small AllToAlls (one per head x s-half, all but the last
overlapped with compute) reshard from head-split to sequence-split for
the output projection. Host reassembles the 8 cores' two row-strips.

Self-contained: hardcodes all shapes; host-side prep is limited to
transpose / dtype-cast / slicing of the inputs.
"""

import sys

import numpy as np

if "/opt/trn_rl_repo" not in sys.path:
    sys.path.insert(0, "/opt/trn_rl_repo")

S, D, A, H = 2048, 1024, 1024, 16
NCORES = 8
HPC = H // NCORES            # heads per core = 2
HD = A // H                  # head dim = 64
E = HPC * HD                 # local ctx rows = 128
ND = D // 128                # d tiles = 8
NT = S // 128                # t tiles = 16
LN2 = 0.6931471805599453
EXP_SCALE = LN2 * (HD ** -0.5)   # p = 2^(score/8) = exp(score * ln2/8)

# attention s-chunking == ReduceScatter chunking
CH = 1024
NCH = S // CH
RSS = CH // NCORES           # rows per core per RS chunk = 128
SS = S // NCORES             # seq slice per core for proj = 256

_CACHE = {}


def _build(enable_asserts=False, debug_taps=False):
    import concourse.bass as bass
    import concourse.tile as tile
    import concourse.mybir as mybir
    from concourse import bacc
    from concourse.masks import make_identity

    f16 = mybir.dt.float16
    f32 = mybir.dt.float32

    nc = bacc.Bacc(
        "TRN2",
        target_bir_lowering=False,
        debug=False,
        enable_asserts=enable_asserts,
        num_devices=NCORES,
    )

    xT = nc.dram_tensor("xT", [D, S], f16, kind="ExternalInput").ap()
    wqT = nc.dram_tensor("wqT", [D, E], f16, kind="ExternalInput").ap()
    wkT = nc.dram_tensor("wkT", [D, E], f16, kind="ExternalInput").ap()
    wvT = nc.dram_tensor("wvT", [D, E], f16, kind="ExternalInput").ap()
    woT = nc.dram_tensor("woT", [A, D], f16, kind="ExternalInput").ap()
    out = nc.dram_tensor("out", [NCH, 128, D], f16, kind="ExternalOutput").ap()
    taps = None
    if debug_taps:
        taps = {
            name: nc.dram_tensor(name, shape, dt, kind="ExternalOutput").ap()
            for name, shape, dt in [
                ("dbg_qT", [128, S], f16),
                ("dbg_kT", [128, S], f16),
                ("dbg_vp", [128, NT * 2 * (HD + 1)], f16),
                ("dbg_pt", [128, CH], f16),
                ("dbg_ctxn0", [HD, S], f16),
                ("dbg_ctxn1", [HD, S], f16),
                ("dbg_outp", [128, D], f16),
            ]
        }

    with tile.TileContext(nc) as tc:
        _body(tc, xT, wqT, wkT, wvT, woT, out, mybir, bass, make_identity, taps)

    nc.compile()
    return nc


def _body(tc, xT, wqT, wkT, wvT, woT, out, mybir, bass, make_identity, taps=None):
    from contextlib import ExitStack

    nc = tc.nc
    f16 = mybir.dt.float16
    f32 = mybir.dt.float32
    Exp = mybir.ActivationFunctionType.Exp

    ctx_stack = ExitStack()
    # ---- persistent SBUF tensors (one bufs=1 pool, distinct tags) ----
    persist = ctx_stack.enter_context(tc.tile_pool(name="persist", bufs=1))

    def ptile(shape, dtype, name):
        return persist.tile(shape, dtype, tag=name, name=name)

    xt_sb = ptile([128, ND, S], f16, "xt_sb")        # x.T, d-tile major
    wq_sb = ptile([128, ND, E], f16, "wq_sb")
    wk_sb = ptile([128, ND, E], f16, "wk_sb")
    wv_sb = ptile([128, ND, E], f16, "wv_sb")
    wo_sb = ptile([128, ND, D], f16, "wo_sb")
    qT_sb = ptile([128, S], f16, "qT_sb")            # [2*hd, s]
    kT_sb = ptile([128, S], f16, "kT_sb")
    vT_sb = ptile([128, S], f16, "vT_sb")
    # v' per t-tile: [v_h0 | ones | v_h1 | ones] -> cols [0:65] and [65:130]
    vp_sb = ptile([128, NT, 2 * (HD + 1)], f16, "vp_sb")
    ident_sb = ptile([128, 128], f16, "ident_sb")
    ones_sb = ptile([HD + 1, HD], f16, "ones_sb")
    # normalized ctx.T per head (base partition 0 each)
    ctxn_h = [ptile([HD, S], f16, f"ctxn_h{h}") for h in range(HPC)]
    ctxf_sb = [
        ptile([128, NCORES, 128], f16, f"ctxf_sb{ci}") for ci in range(NCH)
    ]
    acc_sb = ptile([128, SS // 128, D], f32, "acc_sb")

    make_identity(nc, ident_sb[:])
    nc.vector.memset(ones_sb[:], 1.0)

    # ---- load inputs (batched; xT per d-tile for finer overlap) ----
    for dt_ in range(ND):
        nc.sync.dma_start(wk_sb[:, dt_, :], wkT[dt_ * 128:(dt_ + 1) * 128, :])
        nc.sync.dma_start(wq_sb[:, dt_, :], wqT[dt_ * 128:(dt_ + 1) * 128, :])
        nc.sync.dma_start(wv_sb[:, dt_, :], wvT[dt_ * 128:(dt_ + 1) * 128, :])
        for qq in range(4):
            nc.sync.dma_start(
                xt_sb[:, dt_, qq * 512:(qq + 1) * 512],
                xT[dt_ * 128:(dt_ + 1) * 128, qq * 512:(qq + 1) * 512],
            )
    nc.sync.dma_start(wo_sb[:], woT.rearrange("(a p) d -> p a d", p=128))

    # ---- qkv.T = w.T^T @ x.T : d-tile outer so each weight LDW feeds 4 MMs
    with tc.tile_pool(name="qkv_ps", bufs=2, space="PSUM") as qkv_ps:
        for w_sb, dst in ((wk_sb, kT_sb), (wq_sb, qT_sb), (wv_sb, vT_sb)):
            pss = [
                qkv_ps.tile([128, 512], f32, tag=f"qkv{i}", name=f"qkv{i}")
                for i in range(4)
            ]
            for dt_ in range(ND):
                for sc in range(4):
                    nc.tensor.matmul(
                        pss[sc][:],
                        lhsT=w_sb[:, dt_, :],
                        rhs=xt_sb[:, dt_, sc * 512:(sc + 1) * 512],
                        start=(dt_ == 0),
                        stop=(dt_ == ND - 1),
                    )
            for sc in range(4):
                nc.vector.tensor_copy(dst[:, sc * 512:(sc + 1) * 512], pss[sc][:])

    # ---- v' = v.T transposed back per t-tile, plus ones columns ----
    with tc.tile_pool(name="tr_ps", bufs=3, space="PSUM") as tr_ps:
        for tt in range(NT):
            tp = tr_ps.tile([128, 128], f16, tag="tr")
            nc.tensor.transpose(
                tp[:], vT_sb[:, tt * 128:(tt + 1) * 128], ident_sb[:]
            )
            nc.vector.tensor_copy(vp_sb[:, tt, 0:HD], tp[:, 0:HD])
            nc.vector.tensor_copy(
                vp_sb[:, tt, HD + 1:2 * HD + 1], tp[:, HD:2 * HD]
            )
        nc.vector.memset(vp_sb[:, :, HD:HD + 1], 1.0)
        nc.vector.memset(vp_sb[:, :, 2 * HD + 1:2 * HD + 2], 1.0)

    if taps is not None:
        nc.sync.dma_start(taps["dbg_qT"][:], qT_sb[:])
        nc.sync.dma_start(taps["dbg_kT"][:], kT_sb[:])
        nc.sync.dma_start(taps["dbg_vp"][:], vp_sb[:].rearrange("p a b -> p (a b)"))

    # ---- attention + per-head AllToAll ----
    dram = ctx_stack.enter_context(tc.tile_pool(name="dram", bufs=1, space="DRAM"))
    a2a_in = [
        [
            dram.tile([NCORES, HD, 128], f16, name=f"a2a_in{h}_{ci}")
            for ci in range(NCH)
        ]
        for h in range(HPC)
    ]
    a2a_out = [
        [
            dram.tile([NCORES, HD, 128], f16, name=f"a2a_out{h}_{ci}")
            for ci in range(NCH)
        ]
        for h in range(HPC)
    ]

    with (
        tc.tile_pool(name="sc_ps", bufs=2, space="PSUM") as sc_ps,
        tc.tile_pool(name="ctx_ps", bufs=1, space="PSUM") as ctx_ps,
        tc.tile_pool(name="bc_ps", bufs=2, space="PSUM") as bc_ps,
        tc.tile_pool(name="pt_pool", bufs=4) as pt_pool,
        tc.tile_pool(name="bc_pool", bufs=2) as bc_pool,
        tc.tile_pool(name="den_pool", bufs=2) as den_pool,
    ):
        for h in range(HPC):
            hb = h * HD      # head base partition
            for ci in range(NCH):
                ctx = ctx_ps.tile([HD + 1, CH], f32, tag="ctx", name="ctx")
                for tt in range(NT):
                    sc = sc_ps.tile([128, CH], f32, tag="sc", name="sc")
                    for nn in range(CH // 512):
                        nc.tensor.matmul(
                            sc[:, nn * 512:(nn + 1) * 512],
                            lhsT=kT_sb[hb:hb + HD, tt * 128:(tt + 1) * 128],
                            rhs=qT_sb[hb:hb + HD,
                                      ci * CH + nn * 512:ci * CH + (nn + 1) * 512],
                            start=True,
                            stop=True,
                            tile_position=(hb, 0),
                        )
                    pt = pt_pool.tile([128, CH], f16, tag="pt")
                    nc.scalar.activation(pt[:], sc[:], Exp, scale=EXP_SCALE)
                    if taps is not None and h == 0 and ci == 0 and tt == 0:
                        nc.sync.dma_start(taps["dbg_pt"][:], pt[:])
                    for nn in range(CH // 512):
                        nc.tensor.matmul(
                            ctx[:, nn * 512:(nn + 1) * 512],
                            lhsT=vp_sb[:, tt, h * (HD + 1):(h + 1) * (HD + 1)],
                            rhs=pt[:, nn * 512:(nn + 1) * 512],
                            start=(tt == 0),
                            stop=(tt == NT - 1),
                        )
                # softmax denominator: row HD of ctx psum; normalize and
                # bounce out per 256-wide sub-chunk (= one rank block) so
                # the chain pipelines and nothing big sits on the tail
                for sub in range(CH // SS):
                    r = ci * (CH // SS) + sub
                    s0 = sub * SS
                    den = den_pool.tile([HD + 1, SS], f16, tag="den", name="den")
                    nc.vector.tensor_copy(
                        den[HD:HD + 1, :], ctx[HD:HD + 1, s0:s0 + SS]
                    )
                    # broadcast across partitions via K=1 outer product
                    bcp = bc_ps.tile([HD, SS], f32, tag="bcp", name="bcp")
                    nc.tensor.matmul(
                        bcp[:],
                        lhsT=ones_sb[HD:HD + 1, :],
                        rhs=den[HD:HD + 1, :],
                        start=True,
                        stop=True,
                        tile_position=(HD, 0),
                    )
                    rbc = bc_pool.tile([HD, SS], f32, tag="rbc", name="rbc")
                    nc.vector.reciprocal_approx_fast(rbc[:], bcp[:])
                    nc.vector.tensor_mul(
                        ctxn_h[h][:, r * SS:(r + 1) * SS],
                        ctx[0:HD, s0:s0 + SS],
                        rbc[:],
                    )
                    for half in range(2):
                        blk = 2 * sub + half
                        nc.scalar.dma_start(
                            a2a_in[h][ci][blk],
                            ctxn_h[h][:, ci * CH + blk * 128:
                                       ci * CH + (blk + 1) * 128],
                        )
                nc.gpsimd.collective_compute(
                    "AllToAll",
                    mybir.AluOpType.bypass,
                    replica_groups=[list(range(NCORES))],
                    ins=[a2a_in[h][ci].opt()],
                    outs=[a2a_out[h][ci].opt()],
                )
                for r in range(NCORES):
                    nc.gpsimd.dma_start(
                        ctxf_sb[ci][h * HD:(h + 1) * HD, r, :],
                        a2a_out[h][ci][r],
                    )

        if taps is not None:
            nc.sync.dma_start(taps["dbg_ctxn0"][:], ctxn_h[0][:])
            nc.sync.dma_start(taps["dbg_ctxn1"][:], ctxn_h[1][:])

        # ---- reload: ctxf[:, k, :] rows 0:64 = head-even block k, 64:128 odd ----
        # proj is K-split by head parity: the even-head half (phase A) only
        # needs a2a_out[0], so it runs during the second AllToAll; phase B
        # accumulates the odd-head half on top via SBUF.

        with tc.tile_pool(name="out_pool", bufs=2) as out_pool:
            for ci in range(NCH):
                ob = out_pool.tile([128, D], f16, tag="ob", name="ob")
                for nn in range(2):
                    ps = sc_ps.tile([128, 512], f32, tag="sc", name="proj")
                    for kt in range(ND):
                        nc.tensor.matmul(
                            ps[:],
                            lhsT=ctxf_sb[ci][:, kt, :],
                            rhs=wo_sb[:, kt, nn * 512:(nn + 1) * 512],
                            start=(kt == 0),
                            stop=(kt == ND - 1),
                        )
                    nc.vector.tensor_copy(ob[:, nn * 512:(nn + 1) * 512], ps[:])
                nc.scalar.dma_start(out[ci], ob[:])
                if taps is not None and ci == 0:
                    nc.sync.dma_start(taps["dbg_outp"][:], ob[:])

    ctx_stack.close()


def get_nc(enable_asserts=False, debug_taps=False):
    key = ("nc", enable_asserts, debug_taps)
    if key not in _CACHE:
        _CACHE[key] = _build(enable_asserts, debug_taps)
    return _CACHE[key]


def make_in_maps(x, w_in, w_out):
    x = np.asarray(x, dtype=np.float32)
    w_in = np.asarray(w_in, dtype=np.float32)
    w_out = np.asarray(w_out, dtype=np.float32)
    xT = np.ascontiguousarray(x.T).astype(np.float16)
    w_outT = w_out.T.astype(np.float16)          # [A(e), D]
    in_maps = []
    for c in range(NCORES):
        r0 = c * E
        wq = np.ascontiguousarray(w_in[r0:r0 + E].T).astype(np.float16)
        wk = np.ascontiguousarray(w_in[A + r0:A + r0 + E].T).astype(np.float16)
        wv = np.ascontiguousarray(
            w_in[2 * A + r0:2 * A + r0 + E].T
        ).astype(np.float16)
        in_maps.append(
            {"xT": xT, "wqT": wq, "wkT": wk, "wvT": wv, "woT": w_outT}
        )
    return in_maps


def assemble_out(results):
    """results[c]["out"] is [NCH, 128, D] fp16; strip ci = out rows
    [ci*CH + c*128 : +128]."""
    full = np.empty((S, D), dtype=np.float32)
    for c in range(NCORES):
        o = results[c]["out"]
        for ci in range(NCH):
            r0 = ci * CH + c * 128
            full[r0:r0 + 128] = o[ci].astype(np.float32)
    return full


def kernel(x, w_in, w_out, tgt_len=None, **kwargs):
    from concourse.bass_utils import run_bass_kernel_spmd

    nc = get_nc()
    in_maps = make_in_maps(x, w_in, w_out)
    res = run_bass_kernel_spmd(nc, in_maps, core_ids=list(range(NCORES)))
    return assemble_out(res.results)



# revision 8
# speedup vs baseline: 1.1117x; 1.1117x over previous
"""Multi-headed self-attention (S=2048, D=1024, H=16) on 8 trn2 NeuronCores.

Tensor-parallel over heads (2 heads/core). Restructured for overlap:
 - batched input DMAs (weights first, x per d-tile, w_out last)
 - k/q projections first (8 psum accumulators), then window-pipelined
   attention: chunk c's scores+exp (Act engine) overlap chunk c-1's ctx
   matmuls (PE) with a 1-chunk lag; v-projection and PE-transposes are
   interleaved into window 0's PE slack.
 - engine split: Act = exp only, DVE = copies/normalize, Sync = input
   DMAs + a2a_in writes, GpSimd = collectives/reloads/out DMA.
 - per (head, s-chunk) AllToAll reshards head-split ctx to seq-split for
   the output projection; proj(ci0) is emitted after the last AllToAll
   trigger so it covers the collective wait; warm matmuls hold PE clock.

Self-contained: hardcodes all shapes; host-side prep is limited to
transpose / dtype-cast / slicing of the inputs.
"""

import sys

import numpy as np

if "/opt/trn_rl_repo" not in sys.path:
    sys.path.insert(0, "/opt/trn_rl_repo")

S, D, A, H = 2048, 1024, 1024, 16
NCORES = 8
HPC = H // NCORES            # heads per core = 2
HD = A // H                  # head dim = 64
E = HPC * HD                 # local qkv rows = 128
ND = D // 128                # d tiles = 8
NT = S // 128                # key tiles = 16
LN2 = 0.6931471805599453
EXP_SCALE = LN2 * (HD ** -0.5)   # p = 2^(score/8) = exp(score * ln2/8)

CH = 1024                    # attention s-chunk == AllToAll chunk
NCH = S // CH                # = 2
NMM = 512                    # matmul moving size (hw max 512 elements)

_CACHE = {}


def _build(enable_asserts=False):
    import concourse.bass as bass
    import concourse.tile as tile
    import concourse.mybir as mybir
    from concourse import bacc
    from concourse.masks import make_identity

    f16 = mybir.dt.float16
    f32 = mybir.dt.float32

    nc = bacc.Bacc(
        "TRN2",
        target_bir_lowering=False,
        debug=False,
        enable_asserts=enable_asserts,
        num_devices=NCORES,
    )

    xT = nc.dram_tensor("xT", [D, S], f16, kind="ExternalInput").ap()
    wqT = nc.dram_tensor("wqT", [D, E], f16, kind="ExternalInput").ap()
    wkT = nc.dram_tensor("wkT", [D, E], f16, kind="ExternalInput").ap()
    wvT = nc.dram_tensor("wvT", [D, E], f16, kind="ExternalInput").ap()
    woT = nc.dram_tensor("woT", [A, D], f16, kind="ExternalInput").ap()
    out = nc.dram_tensor("out", [NCH, 128, D], f16, kind="ExternalOutput").ap()

    with tile.TileContext(nc) as tc:
        _body(tc, xT, wqT, wkT, wvT, woT, out, mybir, bass, make_identity)

    nc.compile()
    return nc


def _body(tc, xT, wqT, wkT, wvT, woT, out, mybir, bass, make_identity):
    from contextlib import ExitStack

    nc = tc.nc
    f16 = mybir.dt.float16
    f32 = mybir.dt.float32
    Exp = mybir.ActivationFunctionType.Exp

    ctx_stack = ExitStack()
    persist = ctx_stack.enter_context(tc.tile_pool(name="persist", bufs=1))

    def ptile(shape, dtype, name):
        return persist.tile(shape, dtype, tag=name, name=name)

    xt_sb = ptile([128, ND, S], f16, "xt_sb")        # x.T, d-tile major
    wq_sb = ptile([128, ND, E], f16, "wq_sb")
    wk_sb = ptile([128, ND, E], f16, "wk_sb")
    wv_sb = ptile([128, ND, E], f16, "wv_sb")
    wo_sb = ptile([128, ND, D], f16, "wo_sb")
    qT_sb = ptile([128, S], f16, "qT_sb")            # [2*hd, s]
    kT_sb = ptile([128, S], f16, "kT_sb")
    vT_sb = ptile([128, S], f16, "vT_sb")
    # v' per t-tile: [v_h0 | ones | v_h1 | ones] -> cols [0:65] and [65:130]
    vp_sb = ptile([128, NT, 2 * (HD + 1)], f16, "vp_sb")
    ident_sb = ptile([128, 128], f16, "ident_sb")
    ones_sb = ptile([HD + 1, HD], f16, "ones_sb")
    # normalized ctx.T per head (base partition 0 each)
    ctxn_h = [ptile([HD, S], f16, f"ctxn_h{h}") for h in range(HPC)]
    ctxf_sb = [
        ptile([128, NCORES, 128], f16, f"ctxf_sb{ci}") for ci in range(NCH)
    ]
    dummy_sb = ptile([1, 32], f16, "dummy_sb")
    dummy32a = ptile([1, 32], f32, "dummy32a")
    dummy32b = ptile([1, 32], f32, "dummy32b")

    make_identity(nc, ident_sb[:])
    nc.vector.memset(ones_sb[:], 1.0)
    nc.vector.memset(vp_sb[:, :, HD:HD + 1], 1.0)
    nc.vector.memset(vp_sb[:, :, 2 * HD + 1:2 * HD + 2], 1.0)
    # preload Exp act table + DVE recip uop table during the DMA wait
    nc.scalar.activation(dummy_sb[:], ident_sb[0:1, 0:32], Exp)
    nc.vector.memset(dummy32a[:], 1.0)
    nc.vector.reciprocal_approx_fast(dummy32b[:], dummy32a[:])

    # ---- input loads: weights first (small), x per d-tile, wo last ----
    nc.sync.dma_start(wk_sb[:], wkT.rearrange("(nd p) e -> p nd e", p=128))
    nc.sync.dma_start(wq_sb[:], wqT.rearrange("(nd p) e -> p nd e", p=128))
    nc.sync.dma_start(wv_sb[:], wvT.rearrange("(nd p) e -> p nd e", p=128))
    for dt_ in range(ND):
        nc.sync.dma_start(xt_sb[:, dt_, :], xT[dt_ * 128:(dt_ + 1) * 128, :])
    nc.sync.dma_start(wo_sb[:], woT.rearrange("(a p) d -> p a d", p=128))

    NKQ = S // NMM            # moving chunks for kq proj (2 @ NMM=1024)

    # ---- k/q projections: 2*NKQ psum accumulators over all 8 banks ----
    with tc.tile_pool(name="kq_ps", bufs=1, space="PSUM") as kq_ps:
        acc = {}
        for wname in ("k", "q"):
            for c in range(NKQ):
                acc[(wname, c)] = kq_ps.tile(
                    [128, NMM], f32, tag=f"a{wname}{c}", name=f"a{wname}{c}"
                )
        for dt_ in range(ND):
            for wname, wsb in (("k", wk_sb), ("q", wq_sb)):
                for c in range(NKQ):
                    nc.tensor.matmul(
                        acc[(wname, c)][:],
                        lhsT=wsb[:, dt_, :],
                        rhs=xt_sb[:, dt_, c * NMM:(c + 1) * NMM],
                        start=(dt_ == 0),
                        stop=(dt_ == ND - 1),
                    )
        for c in range(NKQ):
            nc.vector.tensor_copy(kT_sb[:, c * NMM:(c + 1) * NMM], acc[("k", c)][:])
            nc.scalar.copy(qT_sb[:, c * NMM:(c + 1) * NMM], acc[("q", c)][:])

    # ---- attention: chunks with 1-window lag between scores and ctx ----
    chunks = [(0, 0), (1, 0), (0, 1), (1, 1)]   # (h, ci), ci-outer

    dram = ctx_stack.enter_context(tc.tile_pool(name="dram", bufs=1, space="DRAM"))
    a2a_in = [
        dram.tile([NCORES, HD, 128], f16, name=f"a2a_in{c}") for c in range(4)
    ]
    a2a_out = [
        dram.tile([NCORES, HD, 128], f16, name=f"a2a_out{c}") for c in range(4)
    ]

    sc_ps = ctx_stack.enter_context(tc.tile_pool(name="sc_ps", bufs=2, space="PSUM"))
    pt_pool = ctx_stack.enter_context(tc.tile_pool(name="pt_pool", bufs=20))
    misc = ctx_stack.enter_context(tc.tile_pool(name="misc", bufs=2))
    out_pool = ctx_stack.enter_context(tc.tile_pool(name="out_pool", bufs=2))

    pts = {}

    def emit_score(c, tt):
        h, ci = chunks[c]
        hb = h * HD
        sc = sc_ps.tile([128, CH], f32, tag="sc", name="sc")
        for nn in range(CH // NMM):
            nc.tensor.matmul(
                sc[:, nn * NMM:(nn + 1) * NMM],
                lhsT=kT_sb[hb:hb + HD, tt * 128:(tt + 1) * 128],
                rhs=qT_sb[hb:hb + HD,
                          ci * CH + nn * NMM:ci * CH + (nn + 1) * NMM],
                start=True,
                stop=True,
                tile_position=(hb, 0),
            )
        pt = pt_pool.tile([128, CH], f16, tag="pt", name="pt")
        nc.scalar.activation(pt[:], sc[:], Exp, scale=EXP_SCALE)
        pts[(c, tt)] = pt

    def emit_ctx(c, tt, ctx):
        h, ci = chunks[c]
        pt = pts.pop((c, tt))
        for nn in range(CH // NMM):
            nc.tensor.matmul(
                ctx[:, nn * NMM:(nn + 1) * NMM],
                lhsT=vp_sb[:, tt, h * (HD + 1):(h + 1) * (HD + 1)],
                rhs=pt[:, nn * NMM:(nn + 1) * NMM],
                start=(tt == 0),
                stop=(tt == NT - 1),
            )

    # window 0 filler worklist: v-proj (sc-major), v copies, transposes
    def v_work(v_ps):
        vacc = [None] * 4
        for s in range(4):
            for dt_ in range(ND):
                if dt_ == 0:
                    vacc[s] = v_ps.tile([128, 512], f32, tag="v", name=f"vacc{s}")
                yield lambda s=s, dt_=dt_: nc.tensor.matmul(
                    vacc[s][:],
                    lhsT=wv_sb[:, dt_, :],
                    rhs=xt_sb[:, dt_, s * 512:(s + 1) * 512],
                    start=(dt_ == 0),
                    stop=(dt_ == ND - 1),
                )
            yield lambda s=s: nc.vector.tensor_copy(
                vT_sb[:, s * 512:(s + 1) * 512], vacc[s][:]
            )
            for t in range(4 * s, 4 * s + 4):
                def tr(t=t):
                    tp = v_ps.tile([128, 128], f16, tag="v", name="tp")
                    nc.tensor.transpose(
                        tp[:], vT_sb[:, t * 128:(t + 1) * 128], ident_sb[:]
                    )
                    nc.vector.tensor_copy(vp_sb[:, t, 0:HD], tp[:, 0:HD])
                    nc.vector.tensor_copy(
                        vp_sb[:, t, HD + 1:2 * HD + 1], tp[:, HD:2 * HD]
                    )
                yield tr

    with tc.tile_pool(name="v_ps", bufs=4, space="PSUM") as v_ps:
        work = v_work(v_ps)
        done = False
        for tt in range(NT):
            emit_score(0, tt)
            for _ in range(4 if tt < 4 else 3):
                try:
                    next(work)()
                except StopIteration:
                    done = True
                    break
        while not done:
            try:
                next(work)()
            except StopIteration:
                done = True

    attn_ps = ctx_stack.enter_context(tc.tile_pool(name="attn_ps", bufs=1, space="PSUM"))

    def emit_norm_a2a(c):
        h, ci = chunks[c]
        ctx = ctx_tiles[c]
        den = misc.tile([HD + 1, CH], f16, tag="den", name="den")
        nc.vector.tensor_copy(den[HD:HD + 1, :], ctx[HD:HD + 1, :])
        for nn in range(2):
            scr = attn_ps.tile([128, 512], f32, tag="scr", name="scr", bufs=2)
            nc.tensor.matmul(
                scr[0:HD, :],
                lhsT=ones_sb[HD:HD + 1, :],
                rhs=den[HD:HD + 1, nn * 512:(nn + 1) * 512],
                start=True,
                stop=True,
                tile_position=(HD, 0),
            )
            rbc = misc.tile([HD, 512], f32, tag="rbc", name="rbc")
            nc.vector.reciprocal_approx_fast(rbc[:], scr[0:HD, :])
            nc.vector.tensor_mul(
                ctxn_h[h][:, ci * CH + nn * 512:ci * CH + (nn + 1) * 512],
                ctx[0:HD, nn * 512:(nn + 1) * 512],
                rbc[:],
            )
        nc.sync.dma_start(
            a2a_in[c].rearrange("r p s -> p r s"),
            ctxn_h[h][:, ci * CH:(ci + 1) * CH].rearrange(
                "p (r s) -> p r s", r=NCORES
            ),
        )
        nc.gpsimd.collective_compute(
            "AllToAll",
            mybir.AluOpType.bypass,
            replica_groups=[list(range(NCORES))],
            ins=[a2a_in[c].opt()],
            outs=[a2a_out[c].opt()],
        )

    def emit_reload(c):
        h, ci = chunks[c]
        nc.gpsimd.dma_start(
            ctxf_sb[ci][h * HD:(h + 1) * HD, :, :],
            a2a_out[c].rearrange("r p s -> p r s"),
        )

    def emit_proj(ci):
        ob = out_pool.tile([128, D], f16, tag="ob", name="ob")
        for nn in range(2):
            ps = attn_ps.tile([128, 512], f32, tag="scr", name="proj_ps", bufs=2)
            for kt in range(ND):
                nc.tensor.matmul(
                    ps[:],
                    lhsT=ctxf_sb[ci][:, kt, :],
                    rhs=wo_sb[:, kt, nn * 512:(nn + 1) * 512],
                    start=(kt == 0),
                    stop=(kt == ND - 1),
                )
            nc.vector.tensor_copy(ob[:, nn * 512:(nn + 1) * 512], ps[:])
        nc.gpsimd.dma_start(out[ci], ob[:])

    mybir_ = mybir
    ctx_tiles = {}

    # windows 1..3: ctx(c-1) + scores(c); window 4: ctx(3) only
    for c in range(1, 4):
        ctx_tiles[c - 1] = attn_ps.tile([HD + 1, CH], f32, tag="ctx", name="ctx", bufs=1)
        for tt in range(NT):
            emit_ctx(c - 1, tt, ctx_tiles[c - 1])
            emit_score(c, tt)
        emit_norm_a2a(c - 1)
        emit_reload(c - 1)
    ctx_tiles[3] = attn_ps.tile([HD + 1, CH], f32, tag="ctx", name="ctx", bufs=1)
    for tt in range(NT):
        emit_ctx(3, tt, ctx_tiles[3])
    emit_norm_a2a(3)

    # tail: proj(ci0) covers the last AllToAll; warm matmuls hold PE clock
    emit_proj(0)
    for i in range(8):
        warm = attn_ps.tile([HD, 512], f32, tag="ctx", name="warm", bufs=1)
        nc.tensor.matmul(
            warm[:],
            lhsT=ones_sb[HD:HD + 1, :],
            rhs=kT_sb[HD:HD + 1, 0:512],
            start=True,
            stop=True,
            tile_position=(HD, 0),
        )
    emit_reload(3)
    emit_proj(1)

    ctx_stack.close()


def get_nc(enable_asserts=False):
    key = ("nc", enable_asserts)
    if key not in _CACHE:
        _CACHE[key] = _build(enable_asserts)
    return _CACHE[key]


def make_in_maps(x, w_in, w_out):
    x = np.asarray(x, dtype=np.float32)
    w_in = np.asarray(w_in, dtype=np.float32)
    w_out = np.asarray(w_out, dtype=np.float32)
    xT = np.ascontiguousarray(x.T).astype(np.float16)
    w_outT = w_out.T.astype(np.float16)          # [A(e), D]
    in_maps = []
    for c in range(NCORES):
        r0 = c * E
        wq = np.ascontiguousarray(w_in[r0:r0 + E].T).astype(np.float16)
        wk = np.ascontiguousarray(w_in[A + r0:A + r0 + E].T).astype(np.float16)
        wv = np.ascontiguousarray(
            w_in[2 * A + r0:2 * A + r0 + E].T
        ).astype(np.float16)
        in_maps.append(
            {"xT": xT, "wqT": wq, "wkT": wk, "wvT": wv, "woT": w_outT}
        )
    return in_maps


def assemble_out(results):
    """results[c]["out"] is [NCH, 128, D] fp16; strip ci = out rows
    [ci*CH + c*128 : +128]."""
    full = np.empty((S, D), dtype=np.float32)
    for c in range(NCORES):
        o = results[c]["out"]
        for ci in range(NCH):
            r0 = ci * CH + c * 128
            full[r0:r0 + 128] = o[ci].astype(np.float32)
    return full


def kernel(x, w_in, w_out, tgt_len=None, **kwargs):
    from concourse.bass_utils import run_bass_kernel_spmd

    nc = get_nc()
    in_maps = make_in_maps(x, w_in, w_out)
    res = run_bass_kernel_spmd(nc, in_maps, core_ids=list(range(NCORES)))
    return assemble_out(res.results)
